# revision 38
# baseline (speedup 1.0000x reference)
"""Trainium2 Bass/Tile kernel: AudioXMMDiT cross-attention, sharded over 8 NeuronCores.

Sharding: data-parallel over batch (2) x tensor-parallel over heads (4 groups of 8).
Each core computes, for its (batch b, heads h0..h0+7):
    q = x[b] @ Wq_c.T ; per-head RMSNorm
    k,v = context[b] @ {Wk_c,Wv_c}.T  (with the reference's cat(k,v)->(h d j)
          column interleave resolved on the host by permuting weight rows)
    out = softmax(q_n k_n^T / 8) @ v        -> out[b, :, h0*64:(h0+8)*64]

On-chip dataflow (all matmuls bf16 with fp32 PSUM accumulation):
    All DRAM operands are host-packed partition-major ([p, kc, free]) so every
    DMA lands as a few large per-partition descriptors (weights: one 4-32KB
    descriptor per partition; x: one 16KB descriptor per partition per
    2-block pair). Q/K/V projections in natural layout; per-head RMSNorm
    per chunk (ACT square -> DVE bf16 reduce -> DVE magic-constant rsqrt with
    one Newton step -> DVE rescale to bf16); q/k transposed head-pair-major
    via bf16 PE transposes; scores^T via K=64 row-packed matmul pairs; exp on
    ACT over 2-bank [128,2,512] psum tiles (the only ACT table used is the
    exp one - no table reloads); AV with exp tiles as FWL weights and v
    augmented by a ones column so the softmax denominator falls out of the
    same matmuls.

Schedule: DMA issue first (3 rings), kv projection with shared ctx
stationaries, then a software-pipelined block loop where block bq's
projection chunks are interleaved at half-chunk granularity with block
bq-1's score matmuls (so EXP-gated score MMs never starve the in-order PE)
and per-chunk RMSNorm stats recycle PSUM banks promptly.
"""

import os
import sys
from contextlib import ExitStack

import numpy as np

for _p in ("/opt/trn_rl_repo",):
    if os.path.isdir(_p) and _p not in sys.path:
        sys.path.insert(0, _p)

import ml_dtypes  # noqa: E402

import concourse.bacc as bacc  # noqa: E402
import concourse.tile as tile  # noqa: E402
from concourse import bass_utils, mybir  # noqa: E402
from concourse.masks import make_identity  # noqa: E402

P = 128
DIM = 2048
KC = DIM // P  # 16 contraction chunks
QK = 4         # ctx/w quarter granularity (kc chunks per quarter)
NH = 8         # heads per core
NPAIR = NH // 2
D = 64         # head dim
DA = D + 1     # + ones column (softmax denominator)
M = 512        # context length
MC = M // P    # kpos chunks
W = NH * D     # 512 output columns per core
EPS = 1e-6
SMSCALE = float(D) ** -0.5  # 1/8
NCORES = 8

BF = mybir.dt.bfloat16
F32 = mybir.dt.float32
U32 = mybir.dt.uint32
AX = mybir.AxisListType
AF = mybir.ActivationFunctionType
MUL = mybir.AluOpType.mult
SHR = mybir.AluOpType.logical_shift_right
XOR = mybir.AluOpType.bitwise_xor
SUB = mybir.AluOpType.subtract
ADD = mybir.AluOpType.add
# 0xFFFFFFFF - 0x5f3759df (so K - t == NOT(t) - this, avoiding reverse-subtract)
RSQRT_MAGIC_COMP = 0xFFFFFFFF - 0x5F3759DF

QB = 512
QCH = QB // P


def build_nc(n_q=4096):
    NQB = n_q // QB

    nc = bacc.Bacc(None, target_bir_lowering=False)

    # All inputs host-packed partition-major: [p, ...] with large contiguous
    # per-partition runs.
    xT = nc.dram_tensor("xT", (P, NQB * KC * QB), BF, kind="ExternalInput")
    ctxT = nc.dram_tensor("ctxT", (P, KC * M), BF, kind="ExternalInput")
    wqT = nc.dram_tensor("wqT", (P, KC * W), BF, kind="ExternalInput")
    wkT = nc.dram_tensor("wkT", (P, KC * W), BF, kind="ExternalInput")
    wvT = nc.dram_tensor("wvT", (P, KC * W), BF, kind="ExternalInput")
    out = nc.dram_tensor("out", (n_q, W), F32, kind="ExternalOutput")

    xT_r = xT[:].rearrange("p (b kc n) -> p b kc n", b=NQB, kc=KC)
    ctxT_r = ctxT[:].rearrange("p (kc m) -> p kc m", kc=KC)
    wqT_r = wqT[:].rearrange("p (kc w) -> p kc w", kc=KC)
    wkT_r = wkT[:].rearrange("p (kc w) -> p kc w", kc=KC)
    wvT_r = wvT[:].rearrange("p (kc w) -> p kc w", kc=KC)

    with tile.TileContext(nc) as tc, ExitStack() as es:
        consts = es.enter_context(tc.tile_pool(name="consts", bufs=1))
        stats = es.enter_context(tc.tile_pool(name="stats", bufs=3))
        qpsum = es.enter_context(tc.tile_pool(name="qpsum", bufs=2, space="PSUM"))
        spsum = es.enter_context(tc.tile_pool(name="spsum", bufs=2, space="PSUM"))
        apsum = es.enter_context(tc.tile_pool(name="apsum", bufs=2, space="PSUM"))
        xpool = es.enter_context(tc.tile_pool(name="xpool", bufs=3))
        qpool = es.enter_context(tc.tile_pool(name="qpool", bufs=4))
        qtpool = es.enter_context(tc.tile_pool(name="qtpool", bufs=2))
        opool = es.enter_context(tc.tile_pool(name="opool", bufs=8))
        # exp(scores) storage is split: pairs 0,1 are produced and consumed
        # within one iteration (bufs=1); pairs 2,3 are consumed by the next
        # iteration's early AV slots (bufs=2)
        e01pool = es.enter_context(tc.tile_pool(name="e01pool", bufs=1))
        e23pool = es.enter_context(tc.tile_pool(name="e23pool", bufs=2))
        ph1 = es.enter_context(tc.tile_pool(name="ph1", bufs=1))

        # ---------------- DMA issue first (3 rings) -----------------------
        ctx_q = []
        for qq in range(4):
            t = ph1.tile([P, QK, M], BF, name=f"ctxq{qq}")
            nc.gpsimd.dma_start(t, ctxT_r[:, qq * QK:(qq + 1) * QK, :])
            ctx_q.append(t)
        # wk/wv quarters alternate between the sync and scalar rings so all
        # four quarters land before the kv matmul stream reaches them
        wk_q, wv_q = [], []
        for qq in range(4):
            ring = nc.sync if qq % 2 == 0 else nc.scalar
            tk = ph1.tile([P, QK, W], BF, name=f"wkq{qq}")
            ring.dma_start(tk, wkT_r[:, qq * QK:(qq + 1) * QK, :])
            wk_q.append(tk)
            tv = ph1.tile([P, QK, W], BF, name=f"wvq{qq}")
            ring.dma_start(tv, wvT_r[:, qq * QK:(qq + 1) * QK, :])
            wv_q.append(tv)

        x_tiles = {}

        def load_x(b, engine=None):
            eng = engine or nc.gpsimd
            t = xpool.tile([P, KC, QB], BF, name=f"x{b}", tag="x")
            eng.dma_start(t, xT_r[:, b, :, :])
            x_tiles[b] = t

        wq_sb = consts.tile([P, KC, W], BF)
        nc.scalar.dma_start(wq_sb, wqT_r)
        load_x(0, engine=nc.gpsimd)
        if NQB > 1:
            load_x(1, engine=nc.gpsimd)

        # ---------------- constants ---------------------------------------
        cst_sb = consts.tile([P, 2], F32)
        nc.vector.memset(cst_sb[:, 0:1], EPS)
        nc.vector.memset(cst_sb[:, 1:2], 0.0)
        zero_sb = cst_sb[:, 1:2]

        kT_sb = consts.tile([P, NPAIR, M], BF)   # [pair-local 2*64, pair, kpos]
        v_sb = consts.tile([P, MC, NH, DA], BF)  # [kpos, mc, head, d + ones]
        nc.vector.memset(v_sb, 1.0)              # ones column; rest overwritten
        ident = consts.tile([P, P], BF)
        make_identity(nc, ident)

        k_sb = ph1.tile([P, MC, W], BF)          # normalized k, natural layout

        qT_tiles, e01_tiles, e23_tiles, qn_tiles = {}, {}, {}, {}

        # ---------------- helpers -----------------------------------------
        def dve_rsqrt(ss):
            """y = rsqrt(ss/D + EPS) on DVE: magic-constant seed + 2 Newton
            iterations (~1e-5 rel err)."""
            shp = list(ss.shape)
            m = stats.tile(shp, F32, tag="rsq_m")
            nc.vector.tensor_scalar(m, ss, 1.0 / D, EPS, MUL, ADD)
            y = stats.tile(shp, F32, tag="rsq_y")
            nc.vector.tensor_scalar(
                y.bitcast(U32), m.bitcast(U32), 1, 0xFFFFFFFF, SHR, XOR)
            nc.vector.tensor_scalar(
                y.bitcast(U32), y.bitcast(U32), RSQRT_MAGIC_COMP, None, SUB)
            t = stats.tile(shp, F32, tag="rsq_t")
            for _ in range(2):
                nc.vector.tensor_tensor(t, y, y, MUL)
                nc.vector.tensor_tensor(t, t, m, MUL)
                nc.vector.tensor_scalar(t, t, -0.5, 1.5, MUL, ADD)
                nc.vector.tensor_tensor(y, y, t, MUL)
            return y

        qps_tiles = {}

        def qproj_half(b, qc, half):
            """8 of 16 contraction matmuls for q chunk qc."""
            x_sb = x_tiles[b]
            col = qc * P
            if half == 0:
                qps = qpsum.tile([P, W], F32, tag="qps", name=f"qps{b}_{qc}")
                qps_tiles[(b, qc)] = qps
            else:
                qps = qps_tiles[(b, qc)]
            for kc in range(half * 8, half * 8 + 8):
                nc.tensor.matmul(
                    qps, x_sb[:, kc, col:col + P],
                    wq_sb[:, kc, :],
                    start=(kc == 0), stop=(kc == KC - 1))

        def qstats(b, qc):
            """Per-chunk RMSNorm stats; releases the qps PSUM bank."""
            qps = qps_tiles.pop((b, qc))
            sq = stats.tile([P, W], BF, tag="sq")
            nc.scalar.activation(sq, qps, AF.Square, bias=zero_sb)
            ss = stats.tile([P, NH], F32, tag="ss")
            nc.vector.reduce_sum(
                ss, sq[:].rearrange("p (h d) -> p h d", h=NH), axis=AX.X)
            rq = dve_rsqrt(ss)
            q_n = qpool.tile([P, W], BF, name=f"qn{b}_{qc}", tag="qn")
            nc.vector.tensor_tensor(
                q_n[:].rearrange("p (h d) -> p h d", h=NH),
                qps[:].rearrange("p (h d) -> p h d", h=NH),
                rq[:, :, None].to_broadcast([P, NH, D]),
                MUL)
            qn_tiles[(b, qc)] = q_n

        def qtrans_chunk(b, qc):
            """bf16 PE transpose: q_n chunk -> qT (via psum)."""
            q_n = qn_tiles.pop((b, qc))
            tps = qpsum.tile([P, NPAIR, P], BF, tag="qps", name=f"tps{b}_{qc}")
            for pair in range(NPAIR):
                nc.tensor.transpose(
                    tps[:, pair, :], q_n[:, pair * P:(pair + 1) * P], ident)
            nc.vector.tensor_copy(
                out=qT_tiles[b][:, :, qc * P:(qc + 1) * P], in_=tps)

        def scores_mc(b, pair, mc):
            """scores^T for one head pair, one kpos chunk + its exp."""
            qT_sb = qT_tiles[b]
            exp_sb = e01_tiles[b] if pair < 2 else e23_tiles[b]
            hA = 2 * (pair % 2)
            sps = spsum.tile([P, 2, QB], F32, tag="sps")
            nc.tensor.matmul(
                sps[:, 0, :],
                kT_sb[0:D, pair, mc * P:(mc + 1) * P],
                qT_sb[0:D, pair, :],
                start=True, stop=True)
            nc.tensor.matmul(
                sps[:, 1, :],
                kT_sb[D:2 * D, pair, mc * P:(mc + 1) * P],
                qT_sb[D:2 * D, pair, :],
                start=True, stop=True)
            nc.scalar.activation(
                exp_sb[:, hA:hA + 2, mc, :], sps, AF.Exp,
                bias=zero_sb, scale=SMSCALE)

        def av_heads(b, hg, h0, nh, qc):
            """AV for `nh` heads starting at local head h0 of group hg, one
            q chunk; denominator divide and output DMA for that column span."""
            exp_sb = e01_tiles[b] if hg == 0 else e23_tiles[b]
            avps = apsum.tile([P, 4, DA], F32, tag="avps", name="avps")
            avps = avps[:, 0:nh, :]
            for hh in range(h0, h0 + nh):
                h = hg * 4 + hh
                for mc in range(MC):
                    nc.tensor.matmul(
                        avps[:, hh - h0, :],
                        exp_sb[:, hh, mc, qc * P:(qc + 1) * P],
                        v_sb[:, mc, h, :],
                        start=(mc == 0), stop=(mc == MC - 1))
            rec = stats.tile([P, 4], F32, tag="rec", name="rec")
            rec = rec[:, 0:nh]
            nc.vector.reciprocal(rec, avps[:, :, D])
            o_h = opool.tile([P, 4, D], F32, tag="o", name="o_h")
            o_h = o_h[:, 0:nh, :]
            nc.vector.tensor_tensor(
                o_h, avps[:, :, 0:D],
                rec[:, :, None].to_broadcast([P, nh, D]), MUL)
            col = (hg * 4 + h0) * D
            nc.sync.dma_start(
                out[b * QB + qc * P: b * QB + (qc + 1) * P,
                    col:col + nh * D],
                o_h[:].rearrange("p h d -> p (h d)"))

        def av_hg(b, hg, qc):
            av_heads(b, hg, 0, 4, qc)

        # ---------------- Phase 1: K/V projections ------------------------
        # Shared ctx stationaries: each ctx [128,128] chunk feeds one wk and
        # one wv matmul back-to-back. mc pairs (0,1) then (2,3), with the
        # norm/copy chains of the first pair running under the second's MMs.
        kps_t, vps_t = {}, {}

        def knorm_v(mc):
            kp = kps_t.pop(mc)
            vp = vps_t.pop(mc)
            nc.vector.tensor_copy(
                out=v_sb[:, mc, :, 0:D],
                in_=vp[:].rearrange("p (h d) -> p h d", h=NH))
            sqk = stats.tile([P, W], BF, tag="sq")
            nc.scalar.activation(sqk, kp, AF.Square, bias=zero_sb)
            ssk = stats.tile([P, NH], F32, tag="ss")
            nc.vector.reduce_sum(
                ssk, sqk[:].rearrange("p (h d) -> p h d", h=NH), axis=AX.X)
            rk = dve_rsqrt(ssk)
            nc.vector.tensor_tensor(
                k_sb[:, mc, :].rearrange("p (h d) -> p h d", h=NH),
                kp[:].rearrange("p (h d) -> p h d", h=NH),
                rk[:, :, None].to_broadcast([P, NH, D]),
                MUL)

        for grp, mcs in ((0, (0, 1)), (1, (2, 3))):
            for mc in mcs:
                if grp == 0:
                    kps_t[mc] = qpsum.tile(
                        [P, W], F32, tag="qps", name=f"kps{mc}")
                else:
                    kps_t[mc] = spsum.tile(
                        [P, 2, QB], F32, tag="sps", name=f"kps{mc}")[:, 0, :]
                vps_t[mc] = apsum.tile(
                    [P, W], F32, tag="avps", name=f"vps{mc}")
            for kc in range(KC):
                cq = ctx_q[kc // QK][:, kc % QK, :]
                wk_c = wk_q[kc // QK][:, kc % QK, :]
                wv_c = wv_q[kc // QK][:, kc % QK, :]
                for mc in mcs:
                    lhs = cq[:, mc * P:(mc + 1) * P]
                    nc.tensor.matmul(
                        kps_t[mc], lhs, wk_c,
                        start=(kc == 0), stop=(kc == KC - 1))
                    nc.tensor.matmul(
                        vps_t[mc], lhs, wv_c,
                        start=(kc == 0), stop=(kc == KC - 1))
            for mc in mcs:
                knorm_v(mc)

        # qproj(0) + its stats, k/q transposes interleaved per chunk so the
        # PE transposes overlap the DVE stats chains (bf16 PE transpose-mode)
        qT_tiles[0] = qtpool.tile([P, NPAIR, QB], BF, name="qT0", tag="qT")
        for qc in range(QCH):
            qproj_half(0, qc, 0)
            qproj_half(0, qc, 1)
            qstats(0, qc)
            mc = qc
            tps_k = qpsum.tile([P, NPAIR, P], BF, tag="qps", name=f"tpsk{mc}")
            for pair in range(NPAIR):
                nc.tensor.transpose(
                    tps_k[:, pair, :], k_sb[:, mc, pair * P:(pair + 1) * P],
                    ident)
            nc.vector.tensor_copy(
                out=kT_sb[:, :, mc * P:(mc + 1) * P], in_=tps_k)
            if qc >= 1:
                qtrans_chunk(0, qc - 1)
        qtrans_chunk(0, 3)
        e01_tiles[0] = e01pool.tile(
            [P, 4, MC, QB], BF, name="e01_0", tag="e01")
        e23_tiles[0] = e23pool.tile(
            [P, 4, MC, QB], BF, name="e23_0", tag="e23")
        if NQB > 2:
            load_x(2)

        # ---------------- Phase 2: software-pipelined main loop -----------
        # Iteration i interleaves, at half-chunk granularity on the in-order
        # PE stream, block bq=i's projection chunks (+ per-chunk stats and
        # transposes) with block ba=i-1's scores (pair qc in slot qc). AV
        # head-group 0 slots into qc2/qc3 (its pair-0,1 exps complete by
        # qc1); head-group 1 closes the iteration pair-by-pair, pair 2
        # first, so pair 3's exp latency hides behind pair 2's AV matmuls.
        # The last block's pairs 2,3 are pulled into iteration NQB-1 so the
        # EXP-bound tail halves.
        for i in range(1, NQB + 1):
            bq = i if i < NQB else None      # block running its q pipeline
            ba = i - 1                       # block running attention
            last = ba == NQB - 1             # tail: pairs 2,3 already done
            if bq is not None:
                if bq + 2 < NQB:
                    load_x(bq + 2)
                e01_tiles[bq] = e01pool.tile(
                    [P, 4, MC, QB], BF, name=f"e01_{bq}", tag="e01")
                e23_tiles[bq] = e23pool.tile(
                    [P, 4, MC, QB], BF, name=f"e23_{bq}", tag="e23")
                qT_tiles[bq] = qtpool.tile(
                    [P, NPAIR, QB], BF, name=f"qT{bq}", tag="qT")
            for qc in range(QCH):
                if bq is not None:
                    qproj_half(bq, qc, 0)
                if last:
                    p, h = qc // 2, qc % 2
                    scores_mc(ba, p, 2 * h)
                    scores_mc(ba, p, 2 * h + 1)
                else:
                    scores_mc(ba, qc, 0)
                    scores_mc(ba, qc, 1)
                if bq is not None:
                    qproj_half(bq, qc, 1)
                    qstats(bq, qc)
                if not last:
                    scores_mc(ba, qc, 2)
                    scores_mc(ba, qc, 3)
                if bq is not None and qc >= 1:
                    qtrans_chunk(bq, qc - 1)
                if qc >= 2:
                    hg = 1 if last else 0
                    av_hg(ba, hg, 2 * (qc - 2))
                    av_hg(ba, hg, 2 * (qc - 2) + 1)
            if bq is not None:
                qtrans_chunk(bq, 3)
            # closing AV pair-by-pair: first the pair whose exps finished a
            # slot earlier, then the last pair (latency hidden behind it)
            hg = 0 if last else 1
            for qc in range(QCH):
                av_heads(ba, hg, 0, 2, qc)
            for qc in range(QCH):
                av_heads(ba, hg, 2, 2, qc)
            if bq is not None and bq == NQB - 1:
                # pull the last block's pairs 2,3 forward (qT ready now)
                for mc in range(MC):
                    scores_mc(bq, 2, mc)
                for mc in range(MC):
                    scores_mc(bq, 3, mc)
            del qT_tiles[ba], x_tiles[ba], e01_tiles[ba], e23_tiles[ba]

        debug_pools = (consts, stats, xpool, qpool, qtpool, opool,
                       e01pool, e23pool, qpsum, spsum, apsum, ph1)

    if os.environ.get("KDEBUG_POOLS"):
        for pool in debug_pools:
            try:
                print(f"POOL {pool.name}: {pool.kb_per_partition_size()} KB/part"
                      f" bufs={pool.bufs} space={pool.space}")
                for k, meta in pool.tag_meta.items():
                    print("   ", k, meta)
            except Exception as e:
                print("POOL", pool.name, "err", e)

    nc.compile()
    return nc


_NC_CACHE = {}


def _get_nc(n_q=4096):
    if n_q not in _NC_CACHE:
        _NC_CACHE[n_q] = build_nc(n_q)
    return _NC_CACHE[n_q]


def _pack_pkc(a2d):
    """(DIM, free) -> (P, KC*free) partition-major bf16, contiguous."""
    bf = ml_dtypes.bfloat16
    kc, free = KC, a2d.shape[1]
    return np.ascontiguousarray(
        a2d.reshape(kc, P, free).transpose(1, 0, 2).reshape(P, kc * free)
    ).astype(bf)


def make_in_maps(x, context, Wq, Wk, Wv):
    """Host-side shard + weight permutation. Returns one input map per core."""
    bf = ml_dtypes.bfloat16
    x = np.asarray(x)
    context = np.asarray(context)
    n_q = x.shape[1]
    nqb = n_q // QB
    Wkv = np.concatenate([np.asarray(Wk), np.asarray(Wv)], axis=0)  # (4096, 2048)
    # reference: cat(k,v) reshaped (h d j): head h, dim d -> row h*128 + 2d (+1 for v)
    idx = np.arange(32)[:, None] * 128 + 2 * np.arange(64)[None, :]
    Wk_eff = Wkv[idx]       # (32, 64, 2048)
    Wv_eff = Wkv[idx + 1]   # (32, 64, 2048)
    Wq_eff = np.asarray(Wq).reshape(32, 64, 2048)

    # x: (n, DIM) -> [p, blk, kc, n-within]: 8KB contiguous per (p, blk)
    xT = []
    for b in range(x.shape[0]):
        a = x[b].T.reshape(KC, P, nqb, QB).transpose(1, 2, 0, 3)
        xT.append(np.ascontiguousarray(a.reshape(P, nqb * KC * QB)).astype(bf))
    ctxT = [_pack_pkc(context[b].T) for b in range(context.shape[0])]

    in_maps = []
    for c in range(NCORES):
        b, hg = divmod(c, 4)
        hs = slice(hg * NH, (hg + 1) * NH)
        in_maps.append({
            "xT": xT[b],
            "ctxT": ctxT[b],
            "wqT": _pack_pkc(np.ascontiguousarray(Wq_eff[hs].reshape(W, DIM).T)),
            "wkT": _pack_pkc(np.ascontiguousarray(Wk_eff[hs].reshape(W, DIM).T)),
            "wvT": _pack_pkc(np.ascontiguousarray(Wv_eff[hs].reshape(W, DIM).T)),
        })
    return in_maps


def assemble_output(results, n_q=4096, nb=2):
    outp = np.empty((nb, n_q, DIM), np.float32)
    for c in range(NCORES):
        b, hg = divmod(c, 4)
        outp[b, :, hg * W:(hg + 1) * W] = results[c]["out"]
    return outp


def kernel(x, context, Wq, Wk, Wv, **run_kwargs):
    nc = _get_nc(x.shape[1])
    in_maps = make_in_maps(x, context, Wq, Wk, Wv)
    res = bass_utils.run_bass_kernel_spmd(
        nc, in_maps, core_ids=list(range(NCORES)), **run_kwargs)
    out = assemble_output(res.results, n_q=x.shape[1], nb=x.shape[0])
    if run_kwargs:
        kernel.last_result = res
    return out


# revision 39
# speedup vs baseline: 1.0145x; 1.0145x over previous
"""Trainium2 Bass/Tile kernel: AudioXMMDiT cross-attention, sharded over 8 NeuronCores.

Sharding: data-parallel over batch (2) x tensor-parallel over heads (4 groups of 8).
Each core computes, for its (batch b, heads h0..h0+7):
    q = x[b] @ Wq_c.T ; per-head RMSNorm
    k,v = context[b] @ {Wk_c,Wv_c}.T  (with the reference's cat(k,v)->(h d j)
          column interleave resolved on the host by permuting weight rows)
    out = softmax(q_n k_n^T / 8) @ v        -> out[b, :, h0*64:(h0+8)*64]

On-chip dataflow (all matmuls bf16 with fp32 PSUM accumulation):
    All DRAM operands are host-packed partition-major ([p, kc, free]) so every
    DMA lands as a few large per-partition descriptors (weights: one 4-32KB
    descriptor per partition; x: one 16KB descriptor per partition per
    2-block pair). Q/K/V projections in natural layout; per-head RMSNorm
    per chunk (ACT square -> DVE bf16 reduce -> DVE magic-constant rsqrt with
    one Newton step -> DVE rescale to bf16); q/k transposed head-pair-major
    via bf16 PE transposes; scores^T via K=64 row-packed matmul pairs; exp on
    ACT over 2-bank [128,2,512] psum tiles (the only ACT table used is the
    exp one - no table reloads); AV with exp tiles as FWL weights and v
    augmented by a ones column so the softmax denominator falls out of the
    same matmuls.

Schedule: DMA issue first (3 rings), kv projection with shared ctx
stationaries, then a software-pipelined block loop where block bq's
projection chunks are interleaved at half-chunk granularity with block
bq-1's score matmuls (so EXP-gated score MMs never starve the in-order PE)
and per-chunk RMSNorm stats recycle PSUM banks promptly.
"""

import os
import sys
from contextlib import ExitStack

import numpy as np

for _p in ("/opt/trn_rl_repo",):
    if os.path.isdir(_p) and _p not in sys.path:
        sys.path.insert(0, _p)

import ml_dtypes  # noqa: E402

import concourse.bacc as bacc  # noqa: E402
import concourse.tile as tile  # noqa: E402
from concourse import bass_utils, mybir  # noqa: E402
from concourse.masks import make_identity  # noqa: E402

P = 128
DIM = 2048
KC = DIM // P  # 16 contraction chunks
QK = 4         # ctx/w quarter granularity (kc chunks per quarter)
NH = 8         # heads per core
NPAIR = NH // 2
D = 64         # head dim
DA = D + 1     # + ones column (softmax denominator)
M = 512        # context length
MC = M // P    # kpos chunks
W = NH * D     # 512 output columns per core
EPS = 1e-6
SMSCALE = float(D) ** -0.5  # 1/8
NCORES = 8

BF = mybir.dt.bfloat16
F32 = mybir.dt.float32
U32 = mybir.dt.uint32
AX = mybir.AxisListType
AF = mybir.ActivationFunctionType
MUL = mybir.AluOpType.mult
SHR = mybir.AluOpType.logical_shift_right
XOR = mybir.AluOpType.bitwise_xor
SUB = mybir.AluOpType.subtract
ADD = mybir.AluOpType.add
# 0xFFFFFFFF - 0x5f3759df (so K - t == NOT(t) - this, avoiding reverse-subtract)
RSQRT_MAGIC_COMP = 0xFFFFFFFF - 0x5F3759DF

QB = 512
QCH = QB // P


def build_nc(n_q=4096):
    NQB = n_q // QB

    nc = bacc.Bacc(None, target_bir_lowering=False)

    # All inputs host-packed partition-major: [p, ...] with large contiguous
    # per-partition runs.
    xT = nc.dram_tensor("xT", (P, NQB * KC * QB), BF, kind="ExternalInput")
    ctxT = nc.dram_tensor("ctxT", (P, KC * M), BF, kind="ExternalInput")
    wqT = nc.dram_tensor("wqT", (P, KC * W), BF, kind="ExternalInput")
    wkT = nc.dram_tensor("wkT", (P, KC * W), BF, kind="ExternalInput")
    wvT = nc.dram_tensor("wvT", (P, KC * W), BF, kind="ExternalInput")
    out = nc.dram_tensor("out", (n_q, W), F32, kind="ExternalOutput")

    xT_r = xT[:].rearrange("p (b kc n) -> p b kc n", b=NQB, kc=KC)
    ctxT_r = ctxT[:].rearrange("p (kc m) -> p kc m", kc=KC)
    wqT_r = wqT[:].rearrange("p (kc w) -> p kc w", kc=KC)
    wkT_r = wkT[:].rearrange("p (kc w) -> p kc w", kc=KC)
    wvT_r = wvT[:].rearrange("p (kc w) -> p kc w", kc=KC)

    with tile.TileContext(nc) as tc, ExitStack() as es:
        consts = es.enter_context(tc.tile_pool(name="consts", bufs=1))
        stats = es.enter_context(tc.tile_pool(name="stats", bufs=3))
        qpsum = es.enter_context(tc.tile_pool(name="qpsum", bufs=2, space="PSUM"))
        spsum = es.enter_context(tc.tile_pool(name="spsum", bufs=2, space="PSUM"))
        apsum = es.enter_context(tc.tile_pool(name="apsum", bufs=2, space="PSUM"))
        xpool = es.enter_context(tc.tile_pool(name="xpool", bufs=3))
        qpool = es.enter_context(tc.tile_pool(name="qpool", bufs=4))
        qtpool = es.enter_context(tc.tile_pool(name="qtpool", bufs=2))
        opool = es.enter_context(tc.tile_pool(name="opool", bufs=8))
        epool = es.enter_context(tc.tile_pool(name="epool", bufs=1))
        ph1 = es.enter_context(tc.tile_pool(name="ph1", bufs=1))

        # ---------------- DMA issue first (3 rings) -----------------------
        ctx_q = []
        for qq in range(4):
            t = ph1.tile([P, QK, M], BF, name=f"ctxq{qq}")
            nc.gpsimd.dma_start(t, ctxT_r[:, qq * QK:(qq + 1) * QK, :])
            ctx_q.append(t)
        # wk/wv quarters alternate between the sync and scalar rings so all
        # four quarters land before the kv matmul stream reaches them
        wk_q, wv_q = [], []
        for qq in range(4):
            ring = nc.sync if qq % 2 == 0 else nc.scalar
            tk = ph1.tile([P, QK, W], BF, name=f"wkq{qq}")
            ring.dma_start(tk, wkT_r[:, qq * QK:(qq + 1) * QK, :])
            wk_q.append(tk)
            tv = ph1.tile([P, QK, W], BF, name=f"wvq{qq}")
            ring.dma_start(tv, wvT_r[:, qq * QK:(qq + 1) * QK, :])
            wv_q.append(tv)

        x_tiles = {}

        def load_x(b, engine=None):
            eng = engine or nc.gpsimd
            t = xpool.tile([P, KC, QB], BF, name=f"x{b}", tag="x")
            eng.dma_start(t, xT_r[:, b, :, :])
            x_tiles[b] = t

        wq_sb = consts.tile([P, KC, W], BF)
        nc.scalar.dma_start(wq_sb, wqT_r)
        load_x(0, engine=nc.gpsimd)
        if NQB > 1:
            load_x(1, engine=nc.gpsimd)

        # ---------------- constants ---------------------------------------
        cst_sb = consts.tile([P, 2], F32)
        nc.vector.memset(cst_sb[:, 0:1], EPS)
        nc.vector.memset(cst_sb[:, 1:2], 0.0)
        zero_sb = cst_sb[:, 1:2]

        kT_sb = consts.tile([P, NPAIR, M], BF)   # [pair-local 2*64, pair, kpos]
        v_sb = consts.tile([P, MC, NH, DA], BF)  # [kpos, mc, head, d + ones]
        nc.vector.memset(v_sb, 1.0)              # ones column; rest overwritten
        ident = consts.tile([P, P], BF)
        make_identity(nc, ident)

        k_sb = ph1.tile([P, MC, W], BF)          # normalized k, natural layout

        qT_tiles, exp_tiles, qn_tiles = {}, {}, {}

        # ---------------- helpers -----------------------------------------
        def dve_rsqrt(ss):
            """y = rsqrt(ss/D + EPS) on DVE: magic-constant seed + 2 Newton
            iterations (~1e-5 rel err)."""
            shp = list(ss.shape)
            m = stats.tile(shp, F32, tag="rsq_m")
            nc.vector.tensor_scalar(m, ss, 1.0 / D, EPS, MUL, ADD)
            y = stats.tile(shp, F32, tag="rsq_y")
            nc.vector.tensor_scalar(
                y.bitcast(U32), m.bitcast(U32), 1, 0xFFFFFFFF, SHR, XOR)
            nc.vector.tensor_scalar(
                y.bitcast(U32), y.bitcast(U32), RSQRT_MAGIC_COMP, None, SUB)
            t = stats.tile(shp, F32, tag="rsq_t")
            for _ in range(2):
                nc.vector.tensor_tensor(t, y, y, MUL)
                nc.vector.tensor_tensor(t, t, m, MUL)
                nc.vector.tensor_scalar(t, t, -0.5, 1.5, MUL, ADD)
                nc.vector.tensor_tensor(y, y, t, MUL)
            return y

        qps_tiles = {}

        def qproj_half(b, qc, half):
            """8 of 16 contraction matmuls for q chunk qc."""
            x_sb = x_tiles[b]
            col = qc * P
            if half == 0:
                qps = qpsum.tile([P, W], F32, tag="qps", name=f"qps{b}_{qc}")
                qps_tiles[(b, qc)] = qps
            else:
                qps = qps_tiles[(b, qc)]
            for kc in range(half * 8, half * 8 + 8):
                nc.tensor.matmul(
                    qps, x_sb[:, kc, col:col + P],
                    wq_sb[:, kc, :],
                    start=(kc == 0), stop=(kc == KC - 1))

        def qstats(b, qc):
            """Per-chunk RMSNorm stats; releases the qps PSUM bank."""
            qps = qps_tiles.pop((b, qc))
            sq = stats.tile([P, W], BF, tag="sq")
            nc.scalar.activation(sq, qps, AF.Square, bias=zero_sb)
            ss = stats.tile([P, NH], F32, tag="ss")
            nc.vector.reduce_sum(
                ss, sq[:].rearrange("p (h d) -> p h d", h=NH), axis=AX.X)
            rq = dve_rsqrt(ss)
            q_n = qpool.tile([P, W], BF, name=f"qn{b}_{qc}", tag="qn")
            nc.vector.tensor_tensor(
                q_n[:].rearrange("p (h d) -> p h d", h=NH),
                qps[:].rearrange("p (h d) -> p h d", h=NH),
                rq[:, :, None].to_broadcast([P, NH, D]),
                MUL)
            qn_tiles[(b, qc)] = q_n

        def qtrans_chunk(b, qc):
            """bf16 PE transpose: q_n chunk -> qT (via psum)."""
            q_n = qn_tiles.pop((b, qc))
            tps = qpsum.tile([P, NPAIR, P], BF, tag="qps", name=f"tps{b}_{qc}")
            for pair in range(NPAIR):
                nc.tensor.transpose(
                    tps[:, pair, :], q_n[:, pair * P:(pair + 1) * P], ident)
            nc.vector.tensor_copy(
                out=qT_tiles[b][:, :, qc * P:(qc + 1) * P], in_=tps)

        def scores_mc(b, pair, mc):
            """scores^T for one head pair, one kpos chunk + its exp."""
            qT_sb = qT_tiles[b]
            exp_sb = exp_tiles[b]
            hA = 2 * pair
            sps = spsum.tile([P, 2, QB], F32, tag="sps")
            nc.tensor.matmul(
                sps[:, 0, :],
                kT_sb[0:D, pair, mc * P:(mc + 1) * P],
                qT_sb[0:D, pair, :],
                start=True, stop=True)
            nc.tensor.matmul(
                sps[:, 1, :],
                kT_sb[D:2 * D, pair, mc * P:(mc + 1) * P],
                qT_sb[D:2 * D, pair, :],
                start=True, stop=True)
            nc.scalar.activation(
                exp_sb[:, hA:hA + 2, mc, :], sps, AF.Exp,
                bias=zero_sb, scale=SMSCALE)

        def av_hg(b, hg, qc):
            """AV for one 4-head group, one q chunk; denominator divide and
            per-half output DMA."""
            exp_sb = exp_tiles[b]
            avps = apsum.tile([P, 4, DA], F32, tag="avps")
            for hh in range(4):
                h = hg * 4 + hh
                for mc in range(MC):
                    nc.tensor.matmul(
                        avps[:, hh, :],
                        exp_sb[:, h, mc, qc * P:(qc + 1) * P],
                        v_sb[:, mc, h, :],
                        start=(mc == 0), stop=(mc == MC - 1))
            rec = stats.tile([P, 4], F32, tag="rec")
            nc.vector.reciprocal(rec, avps[:, :, D])
            o_h = opool.tile([P, 4, D], F32, tag="o")
            nc.vector.tensor_tensor(
                o_h, avps[:, :, 0:D],
                rec[:, :, None].to_broadcast([P, 4, D]), MUL)
            nc.sync.dma_start(
                out[b * QB + qc * P: b * QB + (qc + 1) * P,
                    hg * 4 * D:(hg + 1) * 4 * D],
                o_h[:].rearrange("p h d -> p (h d)"))

        # ---------------- Phase 1: K/V projections ------------------------
        # Shared ctx stationaries: each ctx [128,128] chunk feeds one wk and
        # one wv matmul back-to-back. mc pairs (0,1) then (2,3), with the
        # norm/copy chains of the first pair running under the second's MMs.
        kps_t, vps_t = {}, {}

        def knorm_v(mc):
            kp = kps_t.pop(mc)
            vp = vps_t.pop(mc)
            nc.vector.tensor_copy(
                out=v_sb[:, mc, :, 0:D],
                in_=vp[:].rearrange("p (h d) -> p h d", h=NH))
            sqk = stats.tile([P, W], BF, tag="sq")
            nc.scalar.activation(sqk, kp, AF.Square, bias=zero_sb)
            ssk = stats.tile([P, NH], F32, tag="ss")
            nc.vector.reduce_sum(
                ssk, sqk[:].rearrange("p (h d) -> p h d", h=NH), axis=AX.X)
            rk = dve_rsqrt(ssk)
            nc.vector.tensor_tensor(
                k_sb[:, mc, :].rearrange("p (h d) -> p h d", h=NH),
                kp[:].rearrange("p (h d) -> p h d", h=NH),
                rk[:, :, None].to_broadcast([P, NH, D]),
                MUL)

        for grp, mcs in ((0, (0, 1)), (1, (2, 3))):
            for mc in mcs:
                if grp == 0:
                    kps_t[mc] = qpsum.tile(
                        [P, W], F32, tag="qps", name=f"kps{mc}")
                else:
                    kps_t[mc] = spsum.tile(
                        [P, 2, QB], F32, tag="sps", name=f"kps{mc}")[:, 0, :]
                vps_t[mc] = apsum.tile(
                    [P, W], F32, tag="avps", name=f"vps{mc}")
            for kc in range(KC):
                cq = ctx_q[kc // QK][:, kc % QK, :]
                wk_c = wk_q[kc // QK][:, kc % QK, :]
                wv_c = wv_q[kc // QK][:, kc % QK, :]
                for mc in mcs:
                    lhs = cq[:, mc * P:(mc + 1) * P]
                    nc.tensor.matmul(
                        kps_t[mc], lhs, wk_c,
                        start=(kc == 0), stop=(kc == KC - 1))
                    nc.tensor.matmul(
                        vps_t[mc], lhs, wv_c,
                        start=(kc == 0), stop=(kc == KC - 1))
            for mc in mcs:
                knorm_v(mc)

        # qproj(0) + its stats
        exp_tiles[0] = epool.tile([P, NH, MC, QB], BF, name="exp0", tag="exp")
        qT_tiles[0] = qtpool.tile([P, NPAIR, QB], BF, name="qT0", tag="qT")
        for qc in range(QCH):
            qproj_half(0, qc, 0)
            qproj_half(0, qc, 1)
            qstats(0, qc)

        # k + q0 transposes (bf16 PE transpose-mode)
        for mc in range(MC):
            tps_k = qpsum.tile([P, NPAIR, P], BF, tag="qps", name=f"tpsk{mc}")
            for pair in range(NPAIR):
                nc.tensor.transpose(
                    tps_k[:, pair, :], k_sb[:, mc, pair * P:(pair + 1) * P],
                    ident)
            nc.vector.tensor_copy(
                out=kT_sb[:, :, mc * P:(mc + 1) * P], in_=tps_k)
            qtrans_chunk(0, mc)
        if NQB > 2:
            load_x(2)

        # ---------------- Phase 2: software-pipelined main loop -----------
        # Iteration i: block bq=i projection pipeline interleaved with block
        # ba=i-1 attention. Scores for pair qc land in slot qc; AV head-group
        # 0 (pairs 0,1) slots into qc>=2 once its exps are ready; head-group 1
        # plus the last transpose close the iteration. The last block's pairs
        # 2,3 are pulled into iteration NQB-1 so the EXP-bound tail halves.
        for i in range(1, NQB + 1):
            bq = i if i < NQB else None      # block running its q pipeline
            ba = i - 1                       # block running attention
            last = ba == NQB - 1             # tail: pairs 2,3 already done
            if bq is not None:
                if bq + 2 < NQB:
                    load_x(bq + 2)
                exp_tiles[bq] = epool.tile(
                    [P, NH, MC, QB], BF, name=f"exp{bq}", tag="exp")
                qT_tiles[bq] = qtpool.tile(
                    [P, NPAIR, QB], BF, name=f"qT{bq}", tag="qT")
            for qc in range(QCH):
                if bq is not None:
                    qproj_half(bq, qc, 0)
                if last:
                    p, h = qc // 2, qc % 2
                    scores_mc(ba, p, 2 * h)
                    scores_mc(ba, p, 2 * h + 1)
                else:
                    scores_mc(ba, qc, 0)
                    scores_mc(ba, qc, 1)
                if bq is not None:
                    qproj_half(bq, qc, 1)
                    qstats(bq, qc)
                if not last:
                    scores_mc(ba, qc, 2)
                    scores_mc(ba, qc, 3)
                if bq is not None and qc >= 1:
                    qtrans_chunk(bq, qc - 1)
                if last and qc >= 2:
                    # hg1 exps (pairs 2,3) were computed last iteration
                    av_hg(ba, 1, 2 * (qc - 2))
                    av_hg(ba, 1, 2 * (qc - 2) + 1)
                elif qc >= 2:
                    # hg0 exps (pairs 0,1) completed by the qc1 slot
                    av_hg(ba, 0, 2 * (qc - 2))
                    av_hg(ba, 0, 2 * (qc - 2) + 1)
            if bq is not None:
                qtrans_chunk(bq, 3)
            for qc in range(QCH):
                av_hg(ba, 1 - int(last), qc)
            if bq is not None and bq == NQB - 1:
                # pull the last block's pairs 2,3 forward (qT ready now)
                for mc in range(MC):
                    scores_mc(bq, 2, mc)
                for mc in range(MC):
                    scores_mc(bq, 3, mc)
            del qT_tiles[ba], exp_tiles[ba], x_tiles[ba]

        debug_pools = (consts, stats, xpool, qpool, qtpool, opool,
                       epool, qpsum, spsum, apsum, ph1)

    if os.environ.get("KDEBUG_POOLS"):
        for pool in debug_pools:
            try:
                print(f"POOL {pool.name}: {pool.kb_per_partition_size()} KB/part"
                      f" bufs={pool.bufs} space={pool.space}")
                for k, meta in pool.tag_meta.items():
                    print("   ", k, meta)
            except Exception as e:
                print("POOL", pool.name, "err", e)

    nc.compile()
    return nc


_NC_CACHE = {}


def _get_nc(n_q=4096):
    if n_q not in _NC_CACHE:
        _NC_CACHE[n_q] = build_nc(n_q)
    return _NC_CACHE[n_q]


def _pack_pkc(a2d):
    """(DIM, free) -> (P, KC*free) partition-major bf16, contiguous."""
    bf = ml_dtypes.bfloat16
    kc, free = KC, a2d.shape[1]
    return np.ascontiguousarray(
        a2d.reshape(kc, P, free).transpose(1, 0, 2).reshape(P, kc * free)
    ).astype(bf)


def make_in_maps(x, context, Wq, Wk, Wv):
    """Host-side shard + weight permutation. Returns one input map per core."""
    bf = ml_dtypes.bfloat16
    x = np.asarray(x)
    context = np.asarray(context)
    n_q = x.shape[1]
    nqb = n_q // QB
    Wkv = np.concatenate([np.asarray(Wk), np.asarray(Wv)], axis=0)  # (4096, 2048)
    # reference: cat(k,v) reshaped (h d j): head h, dim d -> row h*128 + 2d (+1 for v)
    idx = np.arange(32)[:, None] * 128 + 2 * np.arange(64)[None, :]
    Wk_eff = Wkv[idx]       # (32, 64, 2048)
    Wv_eff = Wkv[idx + 1]   # (32, 64, 2048)
    Wq_eff = np.asarray(Wq).reshape(32, 64, 2048)

    # x: (n, DIM) -> [p, blk, kc, n-within]: 8KB contiguous per (p, blk)
    xT = []
    for b in range(x.shape[0]):
        a = x[b].T.reshape(KC, P, nqb, QB).transpose(1, 2, 0, 3)
        xT.append(np.ascontiguousarray(a.reshape(P, nqb * KC * QB)).astype(bf))
    ctxT = [_pack_pkc(context[b].T) for b in range(context.shape[0])]

    in_maps = []
    for c in range(NCORES):
        b, hg = divmod(c, 4)
        hs = slice(hg * NH, (hg + 1) * NH)
        in_maps.append({
            "xT": xT[b],
            "ctxT": ctxT[b],
            "wqT": _pack_pkc(np.ascontiguousarray(Wq_eff[hs].reshape(W, DIM).T)),
            "wkT": _pack_pkc(np.ascontiguousarray(Wk_eff[hs].reshape(W, DIM).T)),
            "wvT": _pack_pkc(np.ascontiguousarray(Wv_eff[hs].reshape(W, DIM).T)),
        })
    return in_maps


def assemble_output(results, n_q=4096, nb=2):
    outp = np.empty((nb, n_q, DIM), np.float32)
    for c in range(NCORES):
        b, hg = divmod(c, 4)
        outp[b, :, hg * W:(hg + 1) * W] = results[c]["out"]
    return outp


def kernel(x, context, Wq, Wk, Wv, **run_kwargs):
    nc = _get_nc(x.shape[1])
    in_maps = make_in_maps(x, context, Wq, Wk, Wv)
    res = bass_utils.run_bass_kernel_spmd(
        nc, in_maps, core_ids=list(range(NCORES)), **run_kwargs)
    out = assemble_output(res.results, n_q=x.shape[1], nb=x.shape[0])
    if run_kwargs:
        kernel.last_result = res
    return out


# revision 45
# speedup vs baseline: 1.0198x; 1.0052x over previous
"""Trainium2 Bass/Tile kernel: AudioXMMDiT cross-attention, sharded over 8 NeuronCores.

Sharding: data-parallel over batch (2) x tensor-parallel over heads (4 groups of 8).
Each core computes, for its (batch b, heads h0..h0+7):
    q = x[b] @ Wq_c.T ; per-head RMSNorm
    k,v = context[b] @ {Wk_c,Wv_c}.T  (with the reference's cat(k,v)->(h d j)
          column interleave resolved on the host by permuting weight rows)
    out = softmax(q_n k_n^T / 8) @ v        -> out[b, :, h0*64:(h0+8)*64]

On-chip dataflow (all matmuls bf16 with fp32 PSUM accumulation):
    All DRAM operands are host-packed partition-major ([p, kc, free]) so every
    DMA lands as a few large per-partition descriptors (weights: one 4-32KB
    descriptor per partition; x: one 16KB descriptor per partition per
    2-block pair). Q/K/V projections in natural layout; per-head RMSNorm
    per chunk (ACT square -> DVE bf16 reduce -> DVE magic-constant rsqrt with
    one Newton step -> DVE rescale to bf16); q/k transposed head-pair-major
    via bf16 PE transposes; scores^T via K=64 row-packed matmul pairs; exp on
    ACT over 2-bank [128,2,512] psum tiles (the only ACT table used is the
    exp one - no table reloads); AV with exp tiles as FWL weights and v
    augmented by a ones column so the softmax denominator falls out of the
    same matmuls.

Schedule: DMA issue first (3 rings), kv projection with shared ctx
stationaries, then a software-pipelined block loop where block bq's
projection chunks are interleaved at half-chunk granularity with block
bq-1's score matmuls (so EXP-gated score MMs never starve the in-order PE)
and per-chunk RMSNorm stats recycle PSUM banks promptly.
"""

import os
import sys
from contextlib import ExitStack

import numpy as np

for _p in ("/opt/trn_rl_repo",):
    if os.path.isdir(_p) and _p not in sys.path:
        sys.path.insert(0, _p)

import ml_dtypes  # noqa: E402

import concourse.bacc as bacc  # noqa: E402
import concourse.tile as tile  # noqa: E402
from concourse import bass_utils, mybir  # noqa: E402
from concourse.masks import make_identity  # noqa: E402

P = 128
DIM = 2048
KC = DIM // P  # 16 contraction chunks
QK = 4         # ctx/w quarter granularity (kc chunks per quarter)
HK = KC // 2   # wq/x half granularity
NH = 8         # heads per core
NPAIR = NH // 2
D = 64         # head dim
DA = D + 1     # + ones column (softmax denominator)
M = 512        # context length
MC = M // P    # kpos chunks
W = NH * D     # 512 output columns per core
EPS = 1e-6
SMSCALE = float(D) ** -0.5  # 1/8
NCORES = 8

BF = mybir.dt.bfloat16
F32 = mybir.dt.float32
U32 = mybir.dt.uint32
AX = mybir.AxisListType
AF = mybir.ActivationFunctionType
MUL = mybir.AluOpType.mult
SHR = mybir.AluOpType.logical_shift_right
XOR = mybir.AluOpType.bitwise_xor
SUB = mybir.AluOpType.subtract
ADD = mybir.AluOpType.add
# 0xFFFFFFFF - 0x5f3759df (so K - t == NOT(t) - this, avoiding reverse-subtract)
RSQRT_MAGIC_COMP = 0xFFFFFFFF - 0x5F3759DF

QB = 512
QCH = QB // P


def build_nc(n_q=4096):
    NQB = n_q // QB

    nc = bacc.Bacc(None, target_bir_lowering=False)

    # All inputs host-packed partition-major: [p, ...] with large contiguous
    # per-partition runs.
    xT = nc.dram_tensor("xT", (P, NQB * KC * QB), BF, kind="ExternalInput")
    ctxT = nc.dram_tensor("ctxT", (P, KC * M), BF, kind="ExternalInput")
    wqT = nc.dram_tensor("wqT", (P, KC * W), BF, kind="ExternalInput")
    wkT = nc.dram_tensor("wkT", (P, KC * W), BF, kind="ExternalInput")
    wvT = nc.dram_tensor("wvT", (P, KC * W), BF, kind="ExternalInput")
    out = nc.dram_tensor("out", (n_q, W), F32, kind="ExternalOutput")

    xT_r = xT[:].rearrange("p (b h kc n) -> p b h kc n", b=NQB, h=2, kc=KC // 2)
    ctxT_r = ctxT[:].rearrange("p (kc m) -> p kc m", kc=KC)
    wqT_r = wqT[:].rearrange("p (kc w) -> p kc w", kc=KC)
    wkT_r = wkT[:].rearrange("p (kc w) -> p kc w", kc=KC)
    wvT_r = wvT[:].rearrange("p (kc w) -> p kc w", kc=KC)

    with tile.TileContext(nc) as tc, ExitStack() as es:
        consts = es.enter_context(tc.tile_pool(name="consts", bufs=1))
        stats = es.enter_context(tc.tile_pool(name="stats", bufs=3))
        qpsum = es.enter_context(tc.tile_pool(name="qpsum", bufs=2, space="PSUM"))
        spsum = es.enter_context(tc.tile_pool(name="spsum", bufs=2, space="PSUM"))
        apsum = es.enter_context(tc.tile_pool(name="apsum", bufs=2, space="PSUM"))
        xpool = es.enter_context(tc.tile_pool(name="xpool", bufs=3))
        qpool = es.enter_context(tc.tile_pool(name="qpool", bufs=4))
        qtpool = es.enter_context(tc.tile_pool(name="qtpool", bufs=2))
        opool = es.enter_context(tc.tile_pool(name="opool", bufs=8))
        epool = es.enter_context(tc.tile_pool(name="epool", bufs=1))
        ph1 = es.enter_context(tc.tile_pool(name="ph1", bufs=1))

        # ---------------- DMA issue first (3 rings, priority-striped) -----
        # Quarter i of wk/wv/ctx lands at the same time on its own ring, so
        # the kv matmul stream is fed just-in-time from t~13us; wq and x0
        # (needed ~45us) trail on the same rings; x1 streams behind ctx.
        ctx_q, wk_q, wv_q = [], [], []
        for qq in range(4):
            tk = ph1.tile([P, QK, W], BF, name=f"wkq{qq}")
            nc.sync.dma_start(tk, wkT_r[:, qq * QK:(qq + 1) * QK, :])
            wk_q.append(tk)
            tv = ph1.tile([P, QK, W], BF, name=f"wvq{qq}")
            nc.scalar.dma_start(tv, wvT_r[:, qq * QK:(qq + 1) * QK, :])
            wv_q.append(tv)
            t = ph1.tile([P, QK, M], BF, name=f"ctxq{qq}")
            nc.gpsimd.dma_start(t, ctxT_r[:, qq * QK:(qq + 1) * QK, :])
            ctx_q.append(t)

        x_tiles = {}

        def load_x(b, engines=None):
            engs = engines or (nc.gpsimd, nc.gpsimd)
            hs = []
            for h in range(2):
                t = xpool.tile([P, HK, QB], BF, name=f"x{b}_{h}", tag="x")
                engs[h].dma_start(t, xT_r[:, b, h, :, :])
                hs.append(t)
            x_tiles[b] = hs

        wq_h = []
        for h, ring in ((0, nc.sync), (1, nc.scalar)):
            t = consts.tile([P, HK, W], BF, name=f"wqh{h}")
            ring.dma_start(t, wqT_r[:, h * HK:(h + 1) * HK, :])
            wq_h.append(t)
        load_x(0, engines=(nc.sync, nc.scalar))
        if NQB > 1:
            load_x(1)

        # ---------------- constants ---------------------------------------
        cst_sb = consts.tile([P, 2], F32)
        nc.vector.memset(cst_sb[:, 0:1], EPS)
        nc.vector.memset(cst_sb[:, 1:2], 0.0)
        zero_sb = cst_sb[:, 1:2]

        kT_sb = consts.tile([P, NPAIR, M], BF)   # [pair-local 2*64, pair, kpos]
        v_sb = consts.tile([P, MC, NH, DA], BF)  # [kpos, mc, head, d + ones]
        nc.vector.memset(v_sb, 1.0)              # ones column; rest overwritten
        ident = consts.tile([P, P], BF)
        make_identity(nc, ident)

        # HAM warm-up: dummy matmuls on the identity while the first input
        # DMAs are in flight, so the kv projection starts at full PE clock
        # (cold MMs run at 1.2 instead of 2.4 GHz). ~70 x ~107ns ends well
        # before the first ctx/wk/wv quarters land.
        wps = qpsum.tile([P, P], F32, tag="qps", name="warm")
        for _ in range(70):
            nc.tensor.matmul(wps, ident, ident, start=True, stop=True)

        k_sb = ph1.tile([P, MC, W], BF)          # normalized k, natural layout

        qT_tiles, exp_tiles, qn_tiles = {}, {}, {}

        # ---------------- helpers -----------------------------------------
        def dve_rsqrt(ss):
            """y = rsqrt(ss/D + EPS) on DVE: magic-constant seed + 2 Newton
            iterations (~1e-5 rel err)."""
            shp = list(ss.shape)
            m = stats.tile(shp, F32, tag="rsq_m")
            nc.vector.tensor_scalar(m, ss, 1.0 / D, EPS, MUL, ADD)
            y = stats.tile(shp, F32, tag="rsq_y")
            nc.vector.tensor_scalar(
                y.bitcast(U32), m.bitcast(U32), 1, 0xFFFFFFFF, SHR, XOR)
            nc.vector.tensor_scalar(
                y.bitcast(U32), y.bitcast(U32), RSQRT_MAGIC_COMP, None, SUB)
            t = stats.tile(shp, F32, tag="rsq_t")
            for _ in range(2):
                nc.vector.tensor_tensor(t, y, y, MUL)
                nc.vector.tensor_tensor(t, t, m, MUL)
                nc.vector.tensor_scalar(t, t, -0.5, 1.5, MUL, ADD)
                nc.vector.tensor_tensor(y, y, t, MUL)
            return y

        qps_tiles = {}

        def qproj_half(b, qc, half):
            """8 of 16 contraction matmuls for q chunk qc."""
            x_sb = x_tiles[b][half]
            wq_sb = wq_h[half]
            col = qc * P
            if half == 0:
                qps = qpsum.tile([P, W], F32, tag="qps", name=f"qps{b}_{qc}")
                qps_tiles[(b, qc)] = qps
            else:
                qps = qps_tiles[(b, qc)]
            for kc in range(HK):
                nc.tensor.matmul(
                    qps, x_sb[:, kc, col:col + P],
                    wq_sb[:, kc, :],
                    start=(half == 0 and kc == 0),
                    stop=(half == 1 and kc == HK - 1))

        def qstats(b, qc):
            """Per-chunk RMSNorm stats; releases the qps PSUM bank."""
            qps = qps_tiles.pop((b, qc))
            sq = stats.tile([P, W], BF, tag="sq")
            nc.scalar.activation(sq, qps, AF.Square, bias=zero_sb)
            ss = stats.tile([P, NH], F32, tag="ss")
            nc.vector.reduce_sum(
                ss, sq[:].rearrange("p (h d) -> p h d", h=NH), axis=AX.X)
            rq = dve_rsqrt(ss)
            q_n = qpool.tile([P, W], BF, name=f"qn{b}_{qc}", tag="qn")
            nc.vector.tensor_tensor(
                q_n[:].rearrange("p (h d) -> p h d", h=NH),
                qps[:].rearrange("p (h d) -> p h d", h=NH),
                rq[:, :, None].to_broadcast([P, NH, D]),
                MUL)
            qn_tiles[(b, qc)] = q_n

        def qtrans_chunk(b, qc):
            """bf16 PE transpose: q_n chunk -> qT (via psum)."""
            q_n = qn_tiles.pop((b, qc))
            tps = qpsum.tile([P, NPAIR, P], BF, tag="qps", name=f"tps{b}_{qc}")
            for pair in range(NPAIR):
                nc.tensor.transpose(
                    tps[:, pair, :], q_n[:, pair * P:(pair + 1) * P], ident)
            nc.vector.tensor_copy(
                out=qT_tiles[b][:, :, qc * P:(qc + 1) * P], in_=tps)

        def scores_mc(b, pair, mc):
            """scores^T for one head pair, one kpos chunk + its exp."""
            qT_sb = qT_tiles[b]
            exp_sb = exp_tiles[b]
            hA = 2 * pair
            sps = spsum.tile([P, 2, QB], F32, tag="sps")
            nc.tensor.matmul(
                sps[:, 0, :],
                kT_sb[0:D, pair, mc * P:(mc + 1) * P],
                qT_sb[0:D, pair, :],
                start=True, stop=True)
            nc.tensor.matmul(
                sps[:, 1, :],
                kT_sb[D:2 * D, pair, mc * P:(mc + 1) * P],
                qT_sb[D:2 * D, pair, :],
                start=True, stop=True)
            nc.scalar.activation(
                exp_sb[:, hA:hA + 2, mc, :], sps, AF.Exp,
                bias=zero_sb, scale=SMSCALE)

        def av_hg(b, hg, qc):
            """AV for one 4-head group, one q chunk; denominator divide and
            per-half output DMA."""
            exp_sb = exp_tiles[b]
            avps = apsum.tile([P, 4, DA], F32, tag="avps")
            for hh in range(4):
                h = hg * 4 + hh
                for mc in range(MC):
                    nc.tensor.matmul(
                        avps[:, hh, :],
                        exp_sb[:, h, mc, qc * P:(qc + 1) * P],
                        v_sb[:, mc, h, :],
                        start=(mc == 0), stop=(mc == MC - 1))
            rec = stats.tile([P, 4], F32, tag="rec")
            nc.vector.reciprocal(rec, avps[:, :, D])
            o_h = opool.tile([P, 4, D], F32, tag="o")
            nc.vector.tensor_tensor(
                o_h, avps[:, :, 0:D],
                rec[:, :, None].to_broadcast([P, 4, D]), MUL)
            nc.sync.dma_start(
                out[b * QB + qc * P: b * QB + (qc + 1) * P,
                    hg * 4 * D:(hg + 1) * 4 * D],
                o_h[:].rearrange("p h d -> p (h d)"))

        # ---------------- Phase 1: K/V projections ------------------------
        # Shared ctx stationaries: each ctx [128,128] chunk feeds one wk and
        # one wv matmul back-to-back. mc pairs (0,1) then (2,3), with the
        # norm/copy chains of the first pair running under the second's MMs.
        kps_t, vps_t = {}, {}

        def knorm_v(mc):
            kp = kps_t.pop(mc)
            vp = vps_t.pop(mc)
            nc.vector.tensor_copy(
                out=v_sb[:, mc, :, 0:D],
                in_=vp[:].rearrange("p (h d) -> p h d", h=NH))
            sqk = stats.tile([P, W], BF, tag="sq")
            nc.scalar.activation(sqk, kp, AF.Square, bias=zero_sb)
            ssk = stats.tile([P, NH], F32, tag="ss")
            nc.vector.reduce_sum(
                ssk, sqk[:].rearrange("p (h d) -> p h d", h=NH), axis=AX.X)
            rk = dve_rsqrt(ssk)
            nc.vector.tensor_tensor(
                k_sb[:, mc, :].rearrange("p (h d) -> p h d", h=NH),
                kp[:].rearrange("p (h d) -> p h d", h=NH),
                rk[:, :, None].to_broadcast([P, NH, D]),
                MUL)

        for grp, mcs in ((0, (0, 1)), (1, (2, 3))):
            for mc in mcs:
                if grp == 0:
                    kps_t[mc] = qpsum.tile(
                        [P, W], F32, tag="qps", name=f"kps{mc}")
                else:
                    kps_t[mc] = spsum.tile(
                        [P, 2, QB], F32, tag="sps", name=f"kps{mc}")[:, 0, :]
                vps_t[mc] = apsum.tile(
                    [P, W], F32, tag="avps", name=f"vps{mc}")
            for kc in range(KC):
                cq = ctx_q[kc // QK][:, kc % QK, :]
                wk_c = wk_q[kc // QK][:, kc % QK, :]
                wv_c = wv_q[kc // QK][:, kc % QK, :]
                for mc in mcs:
                    lhs = cq[:, mc * P:(mc + 1) * P]
                    nc.tensor.matmul(
                        kps_t[mc], lhs, wk_c,
                        start=(kc == 0), stop=(kc == KC - 1))
                    nc.tensor.matmul(
                        vps_t[mc], lhs, wv_c,
                        start=(kc == 0), stop=(kc == KC - 1))
            for mc in mcs:
                knorm_v(mc)

        # qproj(0) + its stats
        exp_tiles[0] = epool.tile([P, NH, MC, QB], BF, name="exp0", tag="exp")
        qT_tiles[0] = qtpool.tile([P, NPAIR, QB], BF, name="qT0", tag="qT")
        for qc in range(QCH):
            qproj_half(0, qc, 0)
            qproj_half(0, qc, 1)
            qstats(0, qc)

        # k + q0 transposes (bf16 PE transpose-mode)
        for mc in range(MC):
            tps_k = qpsum.tile([P, NPAIR, P], BF, tag="qps", name=f"tpsk{mc}")
            for pair in range(NPAIR):
                nc.tensor.transpose(
                    tps_k[:, pair, :], k_sb[:, mc, pair * P:(pair + 1) * P],
                    ident)
            nc.vector.tensor_copy(
                out=kT_sb[:, :, mc * P:(mc + 1) * P], in_=tps_k)
            qtrans_chunk(0, mc)
        if NQB > 2:
            load_x(2)

        # ---------------- Phase 2: software-pipelined main loop -----------
        # Iteration i: block bq=i projection pipeline interleaved with block
        # ba=i-1 attention. Scores for pair qc land in slot qc; AV head-group
        # 0 (pairs 0,1) slots into qc>=2 once its exps are ready; head-group 1
        # plus the last transpose close the iteration. The last block's pairs
        # 2,3 are pulled into iteration NQB-1 so the EXP-bound tail halves.
        for i in range(1, NQB + 1):
            bq = i if i < NQB else None      # block running its q pipeline
            ba = i - 1                       # block running attention
            last = ba == NQB - 1             # tail: pairs 2,3 already done
            if bq is not None:
                if bq + 2 < NQB:
                    load_x(bq + 2)
                exp_tiles[bq] = epool.tile(
                    [P, NH, MC, QB], BF, name=f"exp{bq}", tag="exp")
                qT_tiles[bq] = qtpool.tile(
                    [P, NPAIR, QB], BF, name=f"qT{bq}", tag="qT")
            for qc in range(QCH):
                if bq is not None:
                    qproj_half(bq, qc, 0)
                if last:
                    p, h = qc // 2, qc % 2
                    scores_mc(ba, p, 2 * h)
                    scores_mc(ba, p, 2 * h + 1)
                else:
                    scores_mc(ba, qc, 0)
                    scores_mc(ba, qc, 1)
                if bq is not None:
                    qproj_half(bq, qc, 1)
                    qstats(bq, qc)
                if not last:
                    scores_mc(ba, qc, 2)
                    scores_mc(ba, qc, 3)
                if bq is not None and qc >= 1:
                    qtrans_chunk(bq, qc - 1)
                if last and qc >= 2:
                    # hg1 exps (pairs 2,3) were computed last iteration
                    av_hg(ba, 1, 2 * (qc - 2))
                    av_hg(ba, 1, 2 * (qc - 2) + 1)
                elif qc >= 2:
                    # hg0 exps (pairs 0,1) completed by the qc1 slot
                    av_hg(ba, 0, 2 * (qc - 2))
                    av_hg(ba, 0, 2 * (qc - 2) + 1)
            if bq is not None:
                qtrans_chunk(bq, 3)
            for qc in range(QCH):
                av_hg(ba, 1 - int(last), qc)
            if bq is not None and bq == NQB - 1:
                # pull the last block's pairs 2,3 forward (qT ready now)
                for mc in range(MC):
                    scores_mc(bq, 2, mc)
                for mc in range(MC):
                    scores_mc(bq, 3, mc)
            del qT_tiles[ba], exp_tiles[ba], x_tiles[ba]

        debug_pools = (consts, stats, xpool, qpool, qtpool, opool,
                       epool, qpsum, spsum, apsum, ph1)

    if os.environ.get("KDEBUG_POOLS"):
        for pool in debug_pools:
            try:
                print(f"POOL {pool.name}: {pool.kb_per_partition_size()} KB/part"
                      f" bufs={pool.bufs} space={pool.space}")
                for k, meta in pool.tag_meta.items():
                    print("   ", k, meta)
            except Exception as e:
                print("POOL", pool.name, "err", e)

    nc.compile()
    return nc


_NC_CACHE = {}


def _get_nc(n_q=4096):
    if n_q not in _NC_CACHE:
        _NC_CACHE[n_q] = build_nc(n_q)
    return _NC_CACHE[n_q]


def _pack_pkc(a2d):
    """(DIM, free) -> (P, KC*free) partition-major bf16, contiguous."""
    bf = ml_dtypes.bfloat16
    kc, free = KC, a2d.shape[1]
    return np.ascontiguousarray(
        a2d.reshape(kc, P, free).transpose(1, 0, 2).reshape(P, kc * free)
    ).astype(bf)


def make_in_maps(x, context, Wq, Wk, Wv):
    """Host-side shard + weight permutation. Returns one input map per core."""
    bf = ml_dtypes.bfloat16
    x = np.asarray(x)
    context = np.asarray(context)
    n_q = x.shape[1]
    nqb = n_q // QB
    Wkv = np.concatenate([np.asarray(Wk), np.asarray(Wv)], axis=0)  # (4096, 2048)
    # reference: cat(k,v) reshaped (h d j): head h, dim d -> row h*128 + 2d (+1 for v)
    idx = np.arange(32)[:, None] * 128 + 2 * np.arange(64)[None, :]
    Wk_eff = Wkv[idx]       # (32, 64, 2048)
    Wv_eff = Wkv[idx + 1]   # (32, 64, 2048)
    Wq_eff = np.asarray(Wq).reshape(32, 64, 2048)

    # x: (n, DIM) -> [p, blk, kc, n-within]: 8KB contiguous per (p, blk)
    xT = []
    for b in range(x.shape[0]):
        # (kc p) x (blk n) -> [p, blk, half, kc_in_half, n]
        a = x[b].T.reshape(2, HK, P, nqb, QB).transpose(2, 3, 0, 1, 4)
        xT.append(np.ascontiguousarray(a.reshape(P, nqb * KC * QB)).astype(bf))
    ctxT = [_pack_pkc(context[b].T) for b in range(context.shape[0])]

    in_maps = []
    for c in range(NCORES):
        b, hg = divmod(c, 4)
        hs = slice(hg * NH, (hg + 1) * NH)
        in_maps.append({
            "xT": xT[b],
            "ctxT": ctxT[b],
            "wqT": _pack_pkc(np.ascontiguousarray(Wq_eff[hs].reshape(W, DIM).T)),
            "wkT": _pack_pkc(np.ascontiguousarray(Wk_eff[hs].reshape(W, DIM).T)),
            "wvT": _pack_pkc(np.ascontiguousarray(Wv_eff[hs].reshape(W, DIM).T)),
        })
    return in_maps


def assemble_output(results, n_q=4096, nb=2):
    outp = np.empty((nb, n_q, DIM), np.float32)
    for c in range(NCORES):
        b, hg = divmod(c, 4)
        outp[b, :, hg * W:(hg + 1) * W] = results[c]["out"]
    return outp


def kernel(x, context, Wq, Wk, Wv, **run_kwargs):
    nc = _get_nc(x.shape[1])
    in_maps = make_in_maps(x, context, Wq, Wk, Wv)
    res = bass_utils.run_bass_kernel_spmd(
        nc, in_maps, core_ids=list(range(NCORES)), **run_kwargs)
    out = assemble_output(res.results, n_q=x.shape[1], nb=x.shape[0])
    if run_kwargs:
        kernel.last_result = res
    return out
